# revision 11
# baseline (speedup 1.0000x reference)
"""Trainium2 kernel for nn_ArchitectModule_74036646248739 (GRU plan decoder).

reference:
    context = concat([world_state, goal], -1)          # [B, 3072]
    h0 = context @ W_ctx + b_ctx                       # [B, 1024]
    x0 = broadcast(start_token)                        # [B, 1024]
    64 GRU steps where the output feeds back as input, so x == h
    after the first step.

Strategy: data-parallel over batch: B=512 -> 64 rows/core on 8 cores, no
collectives.  Weights replicated, cast to bf16, resident in SBUF.  Because
x == h for steps >= 2, the r/z gate weights merge: W_rz = W_ih_rz + W_hh_rz,
saving 1/3 of the matmul columns.  Step 1's x-contribution (start_token @
W_ih) is a batch-independent row folded into the step-1 bias on the host.

Per-core step: gates = ones-row-bias-MM + sum_k hT_k.T @ W[k] (PSUM fp32),
sigmoid/tanh on ScalarE, gate algebra on VectorE, h' fed back via PE
transposes (bf16 stationary for the next step's matmuls).

Output layout [T, B_local, D] per core; host reassembles [B, T, D].
"""

import numpy as np
import ml_dtypes

import concourse.bass as bass
import concourse.bacc as bacc
import concourse.mybir as mybir
import concourse.tile as tile
from concourse.bass_utils import run_bass_kernel_spmd
from concourse.masks import make_identity

BF16 = mybir.dt.bfloat16
F32 = mybir.dt.float32
AF = mybir.ActivationFunctionType
ALU = mybir.AluOpType

B, D, T = 512, 1024, 64
NCORES = 8
BL = B // NCORES            # 64 batch rows per core
CTX = 3072
NK = D // 128               # 8 K-tiles over the hidden dim
NKC = CTX // 128            # 24 K-tiles over the context dim
CH = 512                    # matmul moving-operand chunk (one PSUM bank)

_CACHE = {}
TRACE = False
TRACE_KW = {}
LAST_RESULT = [None]
LAST_IN_MAPS = [None]


def _build_nc():
    # Bacc (not plain Bass): its compile() pass legalizes multi-wait
    # instructions (generate_event_semaphores) that Tile emits on DMAs.
    nc = bacc.Bacc("TRN2")

    ctxT_h = nc.declare_dram_parameter("ctxT", [CTX, BL], BF16, isOutput=False)
    wctx_h = nc.declare_dram_parameter("wctx", [CTX, D], BF16, isOutput=False)
    whh_h = nc.declare_dram_parameter("whh", [D, 3 * D], BF16, isOutput=False)
    wall_h = nc.declare_dram_parameter("wall", [D, 4 * D], BF16, isOutput=False)
    bctx_h = nc.declare_dram_parameter("bctx", [1, D], BF16, isOutput=False)
    bias1_h = nc.declare_dram_parameter("bias1", [1, 3 * D], BF16, isOutput=False)
    gin1_h = nc.declare_dram_parameter("gin1", [1, D], F32, isOutput=False)
    biasM_h = nc.declare_dram_parameter("biasM", [1, 4 * D], BF16, isOutput=False)
    out_h = nc.declare_dram_parameter("out", [T, BL, D], F32, isOutput=True)

    with tile.TileContext(nc) as tc:
        with (
            tc.tile_pool(name="wres", bufs=1) as wres,
            tc.tile_pool(name="wstream", bufs=4) as wstream,
            tc.tile_pool(name="consts", bufs=1) as consts,
            tc.tile_pool(name="state", bufs=2) as state,
            tc.tile_pool(name="work", bufs=1) as work,
            tc.tile_pool(name="psum", bufs=1, space="PSUM") as psum,
        ):
            # ---- constants / weights into SBUF ----
            ctxT_sb = consts.tile([128, NKC, BL], BF16)
            nc.sync.dma_start(
                out=ctxT_sb, in_=ctxT_h[:].rearrange("(ko p) b -> p ko b", p=128)
            )
            whh_sb = wres.tile([128, NK, 3 * D], BF16, tag="whh")
            nc.sync.dma_start(
                out=whh_sb, in_=whh_h[:].rearrange("(ko p) n -> p ko n", p=128)
            )
            wall_sb = wres.tile([128, NK, 4 * D], BF16, tag="wall")
            nc.sync.dma_start(
                out=wall_sb, in_=wall_h[:].rearrange("(ko p) n -> p ko n", p=128)
            )
            wctx_t = wctx_h[:].rearrange("(ko p) n -> p ko n", p=128)
            bctx_sb = consts.tile([1, D], BF16)
            nc.sync.dma_start(out=bctx_sb, in_=bctx_h[:])
            bias1_sb = consts.tile([1, 3 * D], BF16)
            nc.sync.dma_start(out=bias1_sb, in_=bias1_h[:])
            biasM_sb = consts.tile([1, 4 * D], BF16)
            nc.sync.dma_start(out=biasM_sb, in_=biasM_h[:])

            # broadcast step-1 n-gate input row to all 64 batch partitions
            gin1_bc = consts.tile([BL, D], F32)
            g1 = gin1_h[:]
            g1_bc = bass.AP(tensor=g1.tensor, offset=g1.offset, ap=[[0, BL], [1, D]])
            nc.gpsimd.dma_start(out=gin1_bc, in_=g1_bc)

            ones_sb = consts.tile([1, BL], BF16)
            nc.vector.memset(ones_sb, 1.0)
            ident_sb = consts.tile([BL, BL], F32)
            make_identity(nc, ident_sb)

            def transpose_h(h_sb, neg_tag=None):
                """h [64, 1024] f32 -> hT [128, NK, 64] bf16 via PE transposes."""
                # shares a slot with pz: pz's last read (sigmoid_z) strictly
                # precedes the transposes, so bufs=1 serializes safely
                ptp = psum.tile([128, CH], F32, tag="pz", name=f"ptp_{nc.next_id()}")
                hT = state.tile(
                    [128, NK, BL], BF16, tag="hT", bufs=2, name=f"hT_{nc.next_id()}"
                )
                for k in range(NK):
                    nc.tensor.transpose(
                        ptp[:, k * BL : (k + 1) * BL],
                        h_sb[:, k * 128 : (k + 1) * 128],
                        ident_sb,
                    )
                    nc.vector.tensor_copy(
                        hT[:, k, :], ptp[:, k * BL : (k + 1) * BL]
                    )
                return hT

            def mm_group(ptile, pcol, hT, w_sb, wcol, bias_sb, bcol):
                """ptile[:, pcol:pcol+CH] = bias-row + sum_k hT_k.T @ w[k]."""
                nc.tensor.matmul(
                    ptile[:, pcol : pcol + CH],
                    ones_sb[0:1, :],
                    bias_sb[0:1, bcol : bcol + CH],
                    start=True,
                    stop=False,
                )
                for k in range(NK):
                    nc.tensor.matmul(
                        ptile[:, pcol : pcol + CH],
                        hT[:, k, :],
                        w_sb[:, k, wcol : wcol + CH],
                        start=False,
                        stop=(k == NK - 1),
                    )

            def gru_step(s, hprev, hT, w_sb, bias_sb, gin_col, ghn_col):
                """One GRU step.  gates laid out in w_sb columns:
                r: 0..1024, z: 1024..2048, gin at gin_col (None for step 1),
                ghn at ghn_col.  Writes out[s]; returns (hnew, hTnew)."""
                i = nc.next_id()
                pr = psum.tile([BL, 2 * CH], F32, tag="pr", name=f"pr_{i}")
                pz = psum.tile([BL, 2 * CH], F32, tag="pz", name=f"pz_{i}")
                pghn = psum.tile([BL, 2 * CH], F32, tag="pghn", name=f"pghn_{i}")
                # r first (its sigmoid is needed earliest), z last (only
                # needed by the final h-update ops)
                mm_group(pr, 0, hT, w_sb, 0, bias_sb, 0)
                mm_group(pr, CH, hT, w_sb, CH, bias_sb, CH)
                mm_group(pghn, 0, hT, w_sb, ghn_col, bias_sb, ghn_col)
                mm_group(pghn, CH, hT, w_sb, ghn_col + CH, bias_sb, ghn_col + CH)
                if gin_col is not None:
                    pgin = psum.tile([BL, 2 * CH], F32, tag="pgin", name=f"pgin_{i}")
                    mm_group(pgin, 0, hT, w_sb, gin_col, bias_sb, gin_col)
                    mm_group(pgin, CH, hT, w_sb, gin_col + CH, bias_sb, gin_col + CH)
                mm_group(pz, 0, hT, w_sb, 2 * CH, bias_sb, 2 * CH)
                mm_group(pz, CH, hT, w_sb, 3 * CH, bias_sb, 3 * CH)

                rs = work.tile([BL, D], BF16, tag="rs", name=f"rs_{i}")
                nc.scalar.activation(rs, pr[:, :], AF.Sigmoid)
                tt = work.tile([BL, D], F32, tag="tt", name=f"tt_{i}")
                nc.vector.tensor_mul(tt, rs, pghn[:, :])
                uu = work.tile([BL, D], F32, tag="uu", name=f"uu_{i}")
                if gin_col is not None:
                    nc.vector.tensor_add(uu, tt, pgin[:, :])
                else:
                    nc.vector.tensor_add(uu, tt, gin1_bc)
                nn_sb = work.tile([BL, D], F32, tag="nn", name=f"nn_{i}")
                nc.scalar.activation(nn_sb, uu, AF.Tanh)
                zs = work.tile([BL, D], BF16, tag="zs", name=f"zs_{i}")
                nc.scalar.activation(zs, pz[:, :], AF.Sigmoid)
                vv = work.tile([BL, D], F32, tag="vv", name=f"vv_{i}")
                nc.vector.tensor_sub(vv, hprev, nn_sb)
                dd = work.tile([BL, D], F32, tag="dd", name=f"dd_{i}")
                # d = (z - 1) * (h - n) = h' - h
                nc.vector.scalar_tensor_tensor(
                    dd, zs, -1.0, vv, op0=ALU.add, op1=ALU.mult
                )
                hnew = state.tile([BL, D], F32, tag="h", bufs=3, name=f"h_{i}")
                nc.vector.tensor_add(hnew, hprev, dd)
                nc.sync.dma_start(out=out_h[s], in_=hnew)
                hTn = transpose_h(hnew)
                return hnew, hTn

            # ---- h0 = context @ W_ctx + b_ctx (W_ctx streamed per K-tile) ----
            ph0 = psum.tile([BL, D], F32, tag="pr")
            for c in range(2):
                nc.tensor.matmul(
                    ph0[:, c * CH : (c + 1) * CH],
                    ones_sb[0:1, :],
                    bctx_sb[0:1, c * CH : (c + 1) * CH],
                    start=True,
                    stop=False,
                )
            for kc in range(NKC):
                wk = wstream.tile([128, D], BF16, tag="wctxk", name=f"wk_{kc}")
                # gpsimd SWDGE: slot-reuse waits exceed the HWDGE wait budget
                nc.gpsimd.dma_start(out=wk, in_=wctx_t[:, kc, :])
                for c in range(2):
                    nc.tensor.matmul(
                        ph0[:, c * CH : (c + 1) * CH],
                        ctxT_sb[:, kc, :],
                        wk[:, c * CH : (c + 1) * CH],
                        start=False,
                        stop=(kc == NKC - 1),
                    )
            h0_sb = state.tile([BL, D], F32, tag="h", bufs=3)
            nc.vector.tensor_copy(h0_sb, ph0[:, :])
            hT = transpose_h(h0_sb)
            hprev = h0_sb

            # ---- step 1: gates = bias1-row + h0 @ W_hh; n-gate x-part is
            # the broadcast gin1 row ----
            hprev, hT = gru_step(0, hprev, hT, whh_sb, bias1_sb, None, 2 * D)

            # ---- steps 2..64 with merged weights ----
            for s in range(1, T):
                hprev, hT = gru_step(s, hprev, hT, wall_sb, biasM_sb, 2 * D, 3 * D)

    nc.finalize()
    return nc


def kernel(world_state, goal, W_ctx, b_ctx, start_token, W_ih, b_ih, W_hh, b_hh):
    bf16 = ml_dtypes.bfloat16
    ws = np.asarray(world_state, dtype=np.float32)
    gl = np.asarray(goal, dtype=np.float32)
    W_ctx = np.asarray(W_ctx, dtype=np.float32)
    b_ctx = np.asarray(b_ctx, dtype=np.float32)
    start_token = np.asarray(start_token, dtype=np.float32)
    W_ih = np.asarray(W_ih, dtype=np.float32)
    b_ih = np.asarray(b_ih, dtype=np.float32)
    W_hh = np.asarray(W_hh, dtype=np.float32)
    b_hh = np.asarray(b_hh, dtype=np.float32)

    if "nc" not in _CACHE:
        _CACHE["nc"] = _build_nc()
    nc = _CACHE["nc"]

    ctxT = np.ascontiguousarray(np.concatenate([ws, gl], axis=1).T)  # [3072, 512]
    ctxT_bf = ctxT.astype(bf16)
    wctx_bf = np.ascontiguousarray(W_ctx).astype(bf16)
    whh_bf = np.ascontiguousarray(W_hh).astype(bf16)
    wall_bf = np.ascontiguousarray(
        np.concatenate(
            [W_ih[:, : 2 * D] + W_hh[:, : 2 * D], W_ih[:, 2 * D :], W_hh[:, 2 * D :]],
            axis=1,
        )
    ).astype(bf16)
    gi1 = start_token @ W_ih + b_ih  # [3072] fp32
    bias1 = np.ascontiguousarray(
        np.concatenate([gi1[: 2 * D] + b_hh[: 2 * D], b_hh[2 * D :]])
    ).astype(bf16)[None]
    gin1 = np.ascontiguousarray(gi1[2 * D :].astype(np.float32))[None]
    biasM = np.ascontiguousarray(
        np.concatenate([b_ih[: 2 * D] + b_hh[: 2 * D], b_ih[2 * D :], b_hh[2 * D :]])
    ).astype(bf16)[None]
    bctx = np.ascontiguousarray(b_ctx).astype(bf16)[None]

    shared = dict(
        wctx=wctx_bf,
        whh=whh_bf,
        wall=wall_bf,
        bctx=bctx,
        bias1=bias1,
        gin1=gin1,
        biasM=biasM,
    )
    in_maps = [
        {**shared, "ctxT": np.ascontiguousarray(ctxT_bf[:, i * BL : (i + 1) * BL])}
        for i in range(NCORES)
    ]

    LAST_IN_MAPS[0] = in_maps
    res = run_bass_kernel_spmd(
        nc, in_maps, core_ids=list(range(NCORES)), trace=TRACE, **TRACE_KW
    )
    LAST_RESULT[0] = res

    full = np.empty((B, T, D), dtype=np.float32)
    for i in range(NCORES):
        o = np.asarray(res.results[i]["out"])  # [T, BL, D]
        full[i * BL : (i + 1) * BL] = o.transpose(1, 0, 2)
    return full
